# revision 8
# baseline (speedup 1.0000x reference)
"""Trainium2 Bass kernel for nn_CrossModalAttention.

Problem: bidirectional cross-attention between two (B, C, H, W) feature maps.
  B=4, C=256, H=W=64 -> N=4096 pixels, HID=64.
  For each direction:  q = Wq@xq, k = Wk@xkv, v = Wv@xkv (1x1 convs),
  attn = softmax_m(q^T k), out = xq + gamma * (v @ attn^T).

Sharding: 2 directions x 4 batches = 8 independent units, one per NeuronCore.

Per-core kernel layout trick: compute S^T tiles [m(part)=128, n(free)=512] via
matmul(lhsT=k_tile, rhs=q_tile) (contraction over HID=64 on partitions), exp on
ScalarE (logits are bounded ~ +-56, so exp in f32 needs no max-subtraction),
then accumulate U[c, n] = sum_m vT[m, c]^T expS^T[m, n] directly in PSUM across
the 32 m-blocks -- no transposes anywhere. Denominator d[n] = sum_m expS^T via
a ones[128,1] matmul accumulated in PSUM. Final: out = xq + (gamma/d)*U + gamma*bv.

Precision: S path (projections + S matmul) in float32r (TF32-like; moving dim
512 >= 256 runs at full PE rate), expS^T/vT in bf16 (measured to contribute
negligible error), all accumulation in f32 PSUM.
"""

import sys

if "/opt/trn_rl_repo" not in sys.path:
    sys.path.insert(0, "/opt/trn_rl_repo")

import numpy as np

B = 4
C = 256
HID = 64
N = 4096          # H*W
P = 128           # SBUF partitions
NT = 512          # n-tile (matmul moving free dim)
N_NT = N // NT    # 8
MB = 128          # m-block (PV contraction tile)
N_MB = N // MB    # 32
CA = C // P       # 2 c-chunks / c-blocks

_CACHE = {}


def _build_program():
    import concourse.bass as bass
    import concourse.mybir as mybir
    from concourse import tile

    f32 = mybir.dt.float32
    f32r = mybir.dt.float32r
    bf16 = mybir.dt.bfloat16
    AF = mybir.ActivationFunctionType

    nc = bass.Bass("TRN2", target_bir_lowering=False, debug=False)

    xq_d = nc.dram_tensor("xq", (C, N), f32r, kind="ExternalInput")
    xkv_d = nc.dram_tensor("xkv", (C, N), f32r, kind="ExternalInput")
    wqT_d = nc.dram_tensor("wqT", (C, HID), f32r, kind="ExternalInput")
    wkT_d = nc.dram_tensor("wkT", (C, HID), f32r, kind="ExternalInput")
    wvT_d = nc.dram_tensor("wvT", (C, C), f32r, kind="ExternalInput")
    bq_d = nc.dram_tensor("bq", (HID, 1), f32, kind="ExternalInput")
    bk_d = nc.dram_tensor("bk", (HID, 1), f32, kind="ExternalInput")
    gbv_d = nc.dram_tensor("gbv", (C, 1), f32, kind="ExternalInput")      # gamma * bv
    rgam_d = nc.dram_tensor("rgam", (1, 1), f32, kind="ExternalInput")    # 1 / gamma
    out_d = nc.dram_tensor("out", (C, N), f32, kind="ExternalOutput")

    # c = a*128 + p views
    xq_r = xq_d[:].rearrange("(a p) n -> p a n", p=P)
    xkv_r = xkv_d[:].rearrange("(a p) n -> p a n", p=P)
    wqT_r = wqT_d[:].rearrange("(a p) h -> p a h", p=P)
    wkT_r = wkT_d[:].rearrange("(a p) h -> p a h", p=P)
    wvT_r = wvT_d[:].rearrange("(a p) c -> p a c", p=P)
    gbv_r = gbv_d[:].rearrange("(a p) one -> p (a one)", p=P)
    out_r = out_d[:].rearrange("(a p) n -> p a n", p=P)

    with tile.TileContext(nc) as tc:
        with (
            tc.tile_pool(name="const", bufs=1) as const,
            tc.tile_pool(name="xin", bufs=1) as xin,
            tc.tile_pool(name="qk", bufs=1) as qk,
            tc.tile_pool(name="vtp", bufs=1) as vtp,
            tc.tile_pool(name="work", bufs=3) as work,
            tc.tile_pool(name="ep", bufs=2) as ep,
            tc.tile_pool(name="dram", bufs=2, space="DRAM") as dram,
            tc.tile_pool(name="psum", bufs=1, space="PSUM") as psum,
        ):
            # ---- constants / weights ----
            wq_sb = const.tile([P, CA, HID], f32r, tag="wq")
            nc.sync.dma_start(wq_sb[:], wqT_r)
            wk_sb = const.tile([P, CA, HID], f32r, tag="wk")
            nc.sync.dma_start(wk_sb[:], wkT_r)
            wv_sb = const.tile([P, CA, C], f32r, tag="wv")
            nc.sync.dma_start(wv_sb[:], wvT_r)
            bq_sb = const.tile([HID, 1], f32, tag="bq")
            nc.sync.dma_start(bq_sb[:], bq_d[:])
            bk_sb = const.tile([HID, 1], f32, tag="bk")
            nc.sync.dma_start(bk_sb[:], bk_d[:])
            gbv_sb = const.tile([P, CA], f32, tag="gbv")
            nc.sync.dma_start(gbv_sb[:], gbv_r)
            rgam_sb = const.tile([1, 1], f32, tag="rgam")
            nc.sync.dma_start(rgam_sb[:], rgam_d[:])
            ones_sb = const.tile([P, 1], bf16, tag="ones")
            nc.vector.memset(ones_sb[:], 1.0)

            # ---- x loads (chunked for DMA/compute overlap) ----
            xq_sb = xin.tile([P, CA, N], f32r, tag="xq")
            xkv_sb = xin.tile([P, CA, N], f32r, tag="xkv")
            NCH = 2048
            for a in range(CA):
                for h in range(N // NCH):
                    sl = slice(h * NCH, (h + 1) * NCH)
                    nc.sync.dma_start(xq_sb[:, a, sl], xq_r[:, a, sl])
                    nc.sync.dma_start(xkv_sb[:, a, sl], xkv_r[:, a, sl])

            # ---- projections ----
            q_sb = qk.tile([HID, N], f32r, tag="q")
            k_sb = qk.tile([HID, N], f32r, tag="k")
            for nt in range(N_NT):
                ntsl = slice(nt * NT, (nt + 1) * NT)
                qp = psum.tile([P, NT], f32, tag="st", bufs=2)
                for a in range(CA):
                    nc.tensor.matmul(
                        qp[:HID, :],
                        lhsT=wq_sb[:, a, :],
                        rhs=xq_sb[:, a, ntsl],
                        start=(a == 0),
                        stop=(a == CA - 1),
                    )
                nc.vector.tensor_scalar_add(q_sb[:, ntsl], qp[:HID, :], bq_sb[:])
                kp = psum.tile([P, NT], f32, tag="st", bufs=2)
                for a in range(CA):
                    nc.tensor.matmul(
                        kp[:HID, :],
                        lhsT=wk_sb[:, a, :],
                        rhs=xkv_sb[:, a, ntsl],
                        start=(a == 0),
                        stop=(a == CA - 1),
                    )
                nc.vector.tensor_scalar_add(k_sb[:, ntsl], kp[:HID, :], bk_sb[:])

            # vT[m, c] = sum_c_in xkv[c_in, m] * WvT[c_in, c], stored bf16
            vt_sb = vtp.tile([P, N_MB, C], bf16, tag="vt")
            for mb in range(N_MB):
                msl = slice(mb * MB, (mb + 1) * MB)
                vp = psum.tile([P, NT], f32, tag="st", bufs=2)
                for a in range(CA):
                    nc.tensor.matmul(
                        vp[:, :C],
                        lhsT=xkv_sb[:, a, msl],
                        rhs=wv_sb[:, a, :],
                        start=(a == 0),
                        stop=(a == CA - 1),
                    )
                nc.vector.tensor_copy(vt_sb[:, mb, :], vp[:, :C])

            # ---- attention ----
            for nt in range(N_NT):
                ntsl = slice(nt * NT, (nt + 1) * NT)
                u0 = psum.tile([P, NT], f32, tag="u", bufs=4, name=f"u0_{nt}")
                u1 = psum.tile([P, NT], f32, tag="u", bufs=4, name=f"u1_{nt}")
                dp = psum.tile([1, NT], f32, tag="dd", bufs=2, name=f"dp_{nt}")
                for mb in range(N_MB):
                    msl = slice(mb * MB, (mb + 1) * MB)
                    stp = psum.tile([P, NT], f32, tag="st", bufs=2, name=f"stp_{nt}_{mb}")
                    nc.tensor.matmul(
                        stp[:],
                        lhsT=k_sb[:, msl],
                        rhs=q_sb[:, ntsl],
                        start=True,
                        stop=True,
                    )
                    ex = work.tile([P, NT], bf16, tag="expst", name=f"ex_{nt}_{mb}")
                    nc.scalar.activation(ex[:], stp[:], AF.Exp)
                    first, last = (mb == 0), (mb == N_MB - 1)
                    nc.tensor.matmul(
                        u0[:], lhsT=vt_sb[:, mb, 0:P], rhs=ex[:], start=first, stop=last
                    )
                    nc.tensor.matmul(
                        u1[:], lhsT=vt_sb[:, mb, P:C], rhs=ex[:], start=first, stop=last
                    )
                    nc.tensor.matmul(
                        dp[:], lhsT=ones_sb[:], rhs=ex[:], start=first, stop=last
                    )
                # epilogue: out[c, n] = xq + (gamma/d[n]) * U[c, n] + gamma*bv[c]
                rd = ep.tile([1, NT], f32, tag="rd", name=f"rd_{nt}")
                nc.vector.tensor_scalar_mul(rd[:], dp[:], rgam_sb[:])   # d/gamma
                grd = ep.tile([1, NT], f32, tag="grd", name=f"grd_{nt}")
                nc.vector.reciprocal(grd[:], rd[:])                     # gamma/d
                # broadcast [1, NT] -> [P, NT] via DRAM roundtrip (DRAM APs
                # allow partition-stride-0 reads; SBUF ones don't)
                dscr = dram.tile([1, NT], f32, tag="dscr", name=f"dscr_{nt}")
                nc.sync.dma_start(dscr[:], grd[:])
                rdb = ep.tile([P, NT], f32, tag="rdb", name=f"rdb_{nt}")
                nc.sync.dma_start(rdb[:], dscr[:].broadcast_to((P, NT)))
                for cb, u in ((0, u0), (1, u1)):
                    t = ep.tile([P, NT], f32, tag="t", name=f"t_{nt}_{cb}")
                    nc.vector.tensor_mul(t[:], u[:], rdb[:])
                    o = ep.tile([P, NT], f32, tag="o", name=f"o_{nt}_{cb}")
                    nc.vector.scalar_tensor_tensor(
                        o[:],
                        in0=t[:],
                        scalar=gbv_sb[:, cb : cb + 1],
                        in1=xq_sb[:, cb, ntsl],
                        op0=mybir.AluOpType.add,
                        op1=mybir.AluOpType.add,
                    )
                    nc.sync.dma_start(out_r[:, cb, ntsl], o[:])

    _split_excess_waits(nc)
    return nc


def _split_excess_waits(nc):
    """The pinned walrus build only encodes 1 sync-wait per instruction;
    newer concourse attaches more. Hoist excess waits onto same-engine NoOps
    inserted immediately before the over-limit instruction (semantically
    identical: same engine, same program position)."""
    import concourse.mybir as mybir
    import bass_rust

    ctr = 0
    for bbl in nc.m.functions[0].blocks:
        il = bbl.instructions
        i = 0
        while i < len(il):
            inst = il[i]
            si = inst.sync_info
            limit = 1
            if si is not None and len(si.on_wait) > limit:
                waits = list(si.on_wait)
                extra = waits[limit:]
                for j in range(0, len(extra), 1):
                    nop = mybir.InstNoOp(name=f"I-wsplit-{ctr}", ins=[], outs=[])
                    ctr += 1
                    nop.engine = inst.engine
                    nop.sync_info = bass_rust.SyncInfo(
                        on_wait=[extra[j]], on_update=[]
                    )
                    il.insert(i, nop)
                    i += 1
                si.on_wait = waits[:limit]
                inst.sync_info = si
            i += 1
    return ctr


def _get_program():
    if "nc" not in _CACHE:
        _CACHE["nc"] = _build_program()
    return _CACHE["nc"]


def _make_in_maps(x1, x2, Wq, bq, Wk, bk, Wv, bv, gamma):
    g = float(np.asarray(gamma).reshape(-1)[0])
    shared = {
        "wqT": np.ascontiguousarray(Wq.T, dtype=np.float32),
        "wkT": np.ascontiguousarray(Wk.T, dtype=np.float32),
        "wvT": np.ascontiguousarray(Wv.T, dtype=np.float32),
        "bq": np.asarray(bq, dtype=np.float32).reshape(HID, 1),
        "bk": np.asarray(bk, dtype=np.float32).reshape(HID, 1),
        "gbv": (g * np.asarray(bv, dtype=np.float32)).reshape(C, 1),
        "rgam": np.array([[1.0 / g if g != 0.0 else 0.0]], dtype=np.float32),
    }
    in_maps = []
    for d in range(2):
        src_q, src_kv = (x1, x2) if d == 0 else (x2, x1)
        for b in range(B):
            in_maps.append(
                {
                    "xq": np.ascontiguousarray(src_q[b].reshape(C, N), dtype=np.float32),
                    "xkv": np.ascontiguousarray(src_kv[b].reshape(C, N), dtype=np.float32),
                    **shared,
                }
            )
    return in_maps


def kernel(x1, x2, Wq, bq, Wk, bk, Wv, bv, gamma, _want_results=False):
    x1 = np.asarray(x1, dtype=np.float32)
    x2 = np.asarray(x2, dtype=np.float32)
    nc = _get_program()
    in_maps = _make_in_maps(x1, x2, Wq, bq, Wk, bk, Wv, bv, gamma)

    from concourse.bass_utils import run_bass_kernel_spmd

    res = run_bass_kernel_spmd(nc, in_maps, core_ids=list(range(2 * B)))
    outs = [r["out"].reshape(C, 64, 64) for r in res.results]
    out1 = np.stack(outs[:B]).astype(np.float32)
    out2 = np.stack(outs[B:]).astype(np.float32)
    if _want_results:
        return (out1, out2), res
    return (out1, out2)


# revision 14
# speedup vs baseline: 1.3595x; 1.3595x over previous
"""Trainium2 Bass kernel for nn_CrossModalAttention.

Problem: bidirectional cross-attention between two (B, C, H, W) feature maps.
  B=4, C=256, H=W=64 -> N=4096 pixels, HID=64.
  For each direction:  q = Wq@xq, k = Wk@xkv, v = Wv@xkv (1x1 convs),
  attn = softmax_m(q^T k), out = xq + gamma * (v @ attn^T).

Sharding: 2 directions x 4 batches = 8 independent units, one per NeuronCore.

Per-core kernel layout trick: compute S^T tiles [m(part)=128, n(free)=512] via
matmul(lhsT=k_tile, rhs=q_tile) (contraction over HID=64 on partitions), exp on
ScalarE (logits are bounded ~ +-56, so exp in f32 needs no max-subtraction),
then accumulate U[c, n] = sum_m vT[m, c]^T expS^T[m, n] directly in PSUM across
the 32 m-blocks -- no transposes anywhere. Denominator d[n] = sum_m expS^T via
a ones[128,1] matmul accumulated in PSUM. Final: out = xq + (gamma/d)*U + gamma*bv.

Precision: S path (projections + S matmul) in float32r (TF32-like; moving dim
512 >= 256 runs at full PE rate), expS^T/vT in bf16 (measured to contribute
negligible error), all accumulation in f32 PSUM.
"""

import sys

if "/opt/trn_rl_repo" not in sys.path:
    sys.path.insert(0, "/opt/trn_rl_repo")

import numpy as np

B = 4
C = 256
HID = 64
N = 4096          # H*W
P = 128           # SBUF partitions
NT = 512          # n-tile (matmul moving free dim)
N_NT = N // NT    # 8
MB = 128          # m-block (PV contraction tile)
N_MB = N // MB    # 32
CA = C // P       # 2 c-chunks / c-blocks

_CACHE = {}


def _build_program():
    import concourse.bass as bass
    import concourse.mybir as mybir
    from concourse import tile

    f32 = mybir.dt.float32
    f32r = mybir.dt.float32r
    bf16 = mybir.dt.bfloat16
    AF = mybir.ActivationFunctionType

    nc = bass.Bass("TRN2", target_bir_lowering=False, debug=False)

    xq_d = nc.dram_tensor("xq", (C, N), f32r, kind="ExternalInput")
    xkv_d = nc.dram_tensor("xkv", (C, N), f32r, kind="ExternalInput")
    wqT_d = nc.dram_tensor("wqT", (C, HID), f32r, kind="ExternalInput")
    wkT_d = nc.dram_tensor("wkT", (C, HID), f32r, kind="ExternalInput")
    wvT_d = nc.dram_tensor("wvT", (C, C), f32r, kind="ExternalInput")
    bq_d = nc.dram_tensor("bq", (HID, 1), f32, kind="ExternalInput")
    bk_d = nc.dram_tensor("bk", (HID, 1), f32, kind="ExternalInput")
    gbv_d = nc.dram_tensor("gbv", (C, 1), f32, kind="ExternalInput")      # gamma * bv
    rgam_d = nc.dram_tensor("rgam", (1, 1), f32, kind="ExternalInput")    # 1 / gamma
    out_d = nc.dram_tensor("out", (C, N), f32, kind="ExternalOutput")

    # c = a*128 + p views
    xq_r = xq_d[:].rearrange("(a p) n -> p a n", p=P)
    xkv_r = xkv_d[:].rearrange("(a p) n -> p a n", p=P)
    wqT_r = wqT_d[:].rearrange("(a p) h -> p a h", p=P)
    wkT_r = wkT_d[:].rearrange("(a p) h -> p a h", p=P)
    wvT_r = wvT_d[:].rearrange("(a p) c -> p a c", p=P)
    gbv_r = gbv_d[:].rearrange("(a p) one -> p (a one)", p=P)
    out_r = out_d[:].rearrange("(a p) n -> p a n", p=P)

    with tile.TileContext(nc) as tc:
        with (
            tc.tile_pool(name="const", bufs=1) as const,
            tc.tile_pool(name="xin", bufs=1) as xin,
            tc.tile_pool(name="qk", bufs=1) as qk,
            tc.tile_pool(name="vtp", bufs=1) as vtp,
            tc.tile_pool(name="work", bufs=3) as work,
            tc.tile_pool(name="ep", bufs=2) as ep,
            tc.tile_pool(name="dram", bufs=2, space="DRAM") as dram,
            tc.tile_pool(name="psum", bufs=1, space="PSUM") as psum,
        ):
            # ---- constants / weights ----
            wq_sb = const.tile([P, CA, HID], f32r, tag="wq")
            nc.sync.dma_start(wq_sb[:], wqT_r)
            wk_sb = const.tile([P, CA, HID], f32r, tag="wk")
            nc.sync.dma_start(wk_sb[:], wkT_r)
            wv_sb = const.tile([P, CA, C], f32r, tag="wv")
            nc.sync.dma_start(wv_sb[:], wvT_r)
            bq_sb = const.tile([HID, 1], f32, tag="bq")
            nc.sync.dma_start(bq_sb[:], bq_d[:])
            bk_sb = const.tile([HID, 1], f32, tag="bk")
            nc.sync.dma_start(bk_sb[:], bk_d[:])
            gbv_sb = const.tile([P, CA], f32, tag="gbv")
            nc.sync.dma_start(gbv_sb[:], gbv_r)
            rgam_sb = const.tile([1, 1], f32, tag="rgam")
            nc.sync.dma_start(rgam_sb[:], rgam_d[:])
            ones_sb = const.tile([P, 1], bf16, tag="ones")
            nc.vector.memset(ones_sb[:], 1.0)

            # ---- x loads (chunked for DMA/compute overlap) ----
            xq_sb = xin.tile([P, CA, N], f32r, tag="xq")
            xkv_sb = xin.tile([P, CA, N], f32r, tag="xkv")
            NCH = 2048
            for a in range(CA):
                for h in range(N // NCH):
                    sl = slice(h * NCH, (h + 1) * NCH)
                    nc.sync.dma_start(xq_sb[:, a, sl], xq_r[:, a, sl])
                    nc.sync.dma_start(xkv_sb[:, a, sl], xkv_r[:, a, sl])

            # ---- projections ----
            # q/k stored twice (rows 0-63 and 64-127) so the K=64 S-matmuls
            # can be row-paired into both halves of the PE array.
            q_sb = qk.tile([P, N], f32r, tag="q")
            k_sb = qk.tile([P, N], f32r, tag="k")
            for nt in range(N_NT):
                ntsl = slice(nt * NT, (nt + 1) * NT)
                qp = psum.tile([P, NT], f32, tag="st", bufs=4)
                for a in range(CA):
                    nc.tensor.matmul(
                        qp[:HID, :],
                        lhsT=wq_sb[:, a, :],
                        rhs=xq_sb[:, a, ntsl],
                        start=(a == 0),
                        stop=(a == CA - 1),
                    )
                nc.vector.tensor_scalar_add(q_sb[0:HID, ntsl], qp[:HID, :], bq_sb[:])
                nc.vector.tensor_scalar_add(q_sb[HID:P, ntsl], qp[:HID, :], bq_sb[:])
                kp = psum.tile([P, NT], f32, tag="st", bufs=4)
                for a in range(CA):
                    nc.tensor.matmul(
                        kp[:HID, :],
                        lhsT=wk_sb[:, a, :],
                        rhs=xkv_sb[:, a, ntsl],
                        start=(a == 0),
                        stop=(a == CA - 1),
                    )
                nc.vector.tensor_scalar_add(k_sb[0:HID, ntsl], kp[:HID, :], bk_sb[:])
                nc.vector.tensor_scalar_add(k_sb[HID:P, ntsl], kp[:HID, :], bk_sb[:])

            # bf16 copies of xkv/Wv for the vT projection (bf16 weight loads
            # get FWL; v precision contributes negligibly to the result)
            wvb_sb = const.tile([P, CA, C], bf16, tag="wvb")
            for a in range(CA):
                nc.vector.tensor_copy(wvb_sb[:, a, :], wv_sb[:, a, :])
            xkvb_sb = xin.tile([P, CA, N], bf16, tag="xkvb")
            for a in range(CA):
                for h in range(N // NCH):
                    sl = slice(h * NCH, (h + 1) * NCH)
                    nc.vector.tensor_copy(xkvb_sb[:, a, sl], xkv_sb[:, a, sl])

            # vT[m, c] = sum_c_in xkv[c_in, m] * WvT[c_in, c], stored bf16
            vt_sb = vtp.tile([P, N_MB, C], bf16, tag="vt")
            for mb in range(N_MB):
                msl = slice(mb * MB, (mb + 1) * MB)
                vp = psum.tile([P, NT], f32, tag="st", bufs=4)
                for a in range(CA):
                    nc.tensor.matmul(
                        vp[:, :C],
                        lhsT=xkvb_sb[:, a, msl],
                        rhs=wvb_sb[:, a, :],
                        start=(a == 0),
                        stop=(a == CA - 1),
                    )
                nc.vector.tensor_copy(vt_sb[:, mb, :], vp[:, :C])

            # ---- attention ----
            for nt in range(N_NT):
                ntsl = slice(nt * NT, (nt + 1) * NT)
                u0 = psum.tile([P, NT], f32, tag="u", bufs=3, name=f"u0_{nt}")
                u1 = psum.tile([P, NT], f32, tag="u", bufs=3, name=f"u1_{nt}")
                dp = psum.tile([1, NT], f32, tag="dd", bufs=1, name=f"dp_{nt}")
                exs = []     # exp tiles awaiting the d sum tree
                n_d = 0      # d-matmuls issued for this n-tile
                for mb in range(N_MB):
                    msl = slice(mb * MB, (mb + 1) * MB)
                    # row-paired S matmul: even m-blocks use PE rows 0-63,
                    # odd ones rows 64-127 (concurrent via tile_position)
                    half = slice(0, HID) if mb % 2 == 0 else slice(HID, P)
                    stp = psum.tile([P, NT], f32, tag="st", bufs=4, name=f"stp_{nt}_{mb}")
                    nc.tensor.matmul(
                        stp[:],
                        lhsT=k_sb[half, msl],
                        rhs=q_sb[half, ntsl],
                        start=True,
                        stop=True,
                    )
                    ex = work.tile([P, NT], bf16, tag="expst", name=f"ex_{nt}_{mb}")
                    nc.scalar.activation(ex[:], stp[:], AF.Exp)
                    first, last = (mb == 0), (mb == N_MB - 1)
                    nc.tensor.matmul(
                        u0[:], lhsT=vt_sb[:, mb, 0:P], rhs=ex[:], start=first, stop=last
                    )
                    nc.tensor.matmul(
                        u1[:], lhsT=vt_sb[:, mb, P:C], rhs=ex[:], start=first, stop=last
                    )
                    # denominator: bf16 binary-counter sum tree on DVE; one
                    # ones-matmul per 8 m-blocks accumulated into dp
                    t_, lvl = ex, 0
                    while exs and exs[-1][0] == lvl:
                        _, prev = exs.pop()
                        s_ = work.tile(
                            [P, NT], bf16, tag=f"dsum{lvl}", bufs=2,
                            name=f"ds_{nt}_{mb}_{lvl}",
                        )
                        nc.vector.tensor_add(s_[:], prev[:], t_[:])
                        t_, lvl = s_, lvl + 1
                    exs.append((lvl, t_))
                    if (mb + 1) % 8 == 0:
                        lvl8, a8 = exs.pop()
                        assert lvl8 == 3 and not exs
                        n_d += 1
                        nc.tensor.matmul(
                            dp[:], lhsT=ones_sb[:], rhs=a8[:],
                            start=(n_d == 1), stop=(n_d == N_MB // 8),
                        )
                # epilogue: out[c, n] = xq + (gamma/d[n]) * U[c, n] + gamma*bv[c]
                rd = ep.tile([1, NT], f32, tag="rd", name=f"rd_{nt}")
                nc.vector.tensor_scalar_mul(rd[:], dp[:], rgam_sb[:])   # d/gamma
                grd = ep.tile([1, NT], f32, tag="grd", name=f"grd_{nt}")
                nc.vector.reciprocal(grd[:], rd[:])                     # gamma/d
                # broadcast [1, NT] -> [P, NT] via DRAM roundtrip (DRAM APs
                # allow partition-stride-0 reads; SBUF ones don't)
                dscr = dram.tile([1, NT], f32, tag="dscr", name=f"dscr_{nt}")
                nc.sync.dma_start(dscr[:], grd[:])
                rdb = ep.tile([P, NT], f32, tag="rdb", name=f"rdb_{nt}")
                nc.sync.dma_start(rdb[:], dscr[:].broadcast_to((P, NT)))
                for cb, u in ((0, u0), (1, u1)):
                    t = ep.tile([P, NT], f32, tag="t", name=f"t_{nt}_{cb}")
                    nc.vector.tensor_mul(t[:], u[:], rdb[:])
                    o = ep.tile([P, NT], f32, tag="o", name=f"o_{nt}_{cb}")
                    nc.vector.scalar_tensor_tensor(
                        o[:],
                        in0=t[:],
                        scalar=gbv_sb[:, cb : cb + 1],
                        in1=xq_sb[:, cb, ntsl],
                        op0=mybir.AluOpType.add,
                        op1=mybir.AluOpType.add,
                    )
                    nc.sync.dma_start(out_r[:, cb, ntsl], o[:])

    return nc


def _split_excess_waits(nc):
    """The pinned walrus build only encodes 1 sync-wait per instruction;
    newer concourse attaches more. Hoist excess waits onto same-engine NoOps
    inserted immediately before the over-limit instruction (semantically
    identical: same engine, same program position)."""
    import concourse.mybir as mybir
    import bass_rust

    ctr = 0
    for bbl in nc.m.functions[0].blocks:
        il = bbl.instructions
        i = 0
        while i < len(il):
            inst = il[i]
            si = inst.sync_info
            limit = 1
            if si is not None and len(si.on_wait) > limit:
                waits = list(si.on_wait)
                extra = waits[limit:]
                for j in range(0, len(extra), 1):
                    nop = mybir.InstNoOp(name=f"I-wsplit-{ctr}", ins=[], outs=[])
                    ctr += 1
                    nop.engine = inst.engine
                    nop.sync_info = bass_rust.SyncInfo(
                        on_wait=[extra[j]], on_update=[]
                    )
                    il.insert(i, nop)
                    i += 1
                si.on_wait = waits[:limit]
                inst.sync_info = si
            i += 1
    return ctr


def _get_program():
    if "nc" not in _CACHE:
        _CACHE["nc"] = _build_program()
    return _CACHE["nc"]


def _get_program_hw():
    """Program with the walrus sync-wait workaround applied (breaks CoreSim's
    race detector, so only applied for hardware runs)."""
    nc = _get_program()
    if not _CACHE.get("split_done"):
        _split_excess_waits(nc)
        _CACHE["split_done"] = True
    return nc


def _make_in_maps(x1, x2, Wq, bq, Wk, bk, Wv, bv, gamma):
    g = float(np.asarray(gamma).reshape(-1)[0])
    shared = {
        "wqT": np.ascontiguousarray(Wq.T, dtype=np.float32),
        "wkT": np.ascontiguousarray(Wk.T, dtype=np.float32),
        "wvT": np.ascontiguousarray(Wv.T, dtype=np.float32),
        "bq": np.asarray(bq, dtype=np.float32).reshape(HID, 1),
        "bk": np.asarray(bk, dtype=np.float32).reshape(HID, 1),
        "gbv": (g * np.asarray(bv, dtype=np.float32)).reshape(C, 1),
        "rgam": np.array([[1.0 / g if g != 0.0 else 0.0]], dtype=np.float32),
    }
    in_maps = []
    for d in range(2):
        src_q, src_kv = (x1, x2) if d == 0 else (x2, x1)
        for b in range(B):
            in_maps.append(
                {
                    "xq": np.ascontiguousarray(src_q[b].reshape(C, N), dtype=np.float32),
                    "xkv": np.ascontiguousarray(src_kv[b].reshape(C, N), dtype=np.float32),
                    **shared,
                }
            )
    return in_maps


def kernel(x1, x2, Wq, bq, Wk, bk, Wv, bv, gamma, _want_results=False):
    x1 = np.asarray(x1, dtype=np.float32)
    x2 = np.asarray(x2, dtype=np.float32)
    nc = _get_program_hw()
    in_maps = _make_in_maps(x1, x2, Wq, bq, Wk, bk, Wv, bv, gamma)

    from concourse.bass_utils import run_bass_kernel_spmd

    res = run_bass_kernel_spmd(nc, in_maps, core_ids=list(range(2 * B)))
    outs = [r["out"].reshape(C, 64, 64) for r in res.results]
    out1 = np.stack(outs[:B]).astype(np.float32)
    out2 = np.stack(outs[B:]).astype(np.float32)
    if _want_results:
        return (out1, out2), res
    return (out1, out2)


# revision 16
# speedup vs baseline: 1.4067x; 1.0347x over previous
"""Trainium2 Bass kernel for nn_CrossModalAttention.

Problem: bidirectional cross-attention between two (B, C, H, W) feature maps.
  B=4, C=256, H=W=64 -> N=4096 pixels, HID=64.
  For each direction:  q = Wq@xq, k = Wk@xkv, v = Wv@xkv (1x1 convs),
  attn = softmax_m(q^T k), out = xq + gamma * (v @ attn^T).

Sharding: 2 directions x 4 batches = 8 independent units, one per NeuronCore.

Per-core kernel layout trick: compute S^T tiles [m(part)=128, n(free)=512] via
matmul(lhsT=k_tile, rhs=q_tile) (contraction over HID=64 on partitions), exp on
ScalarE (logits are bounded ~ +-56, so exp in f32 needs no max-subtraction),
then accumulate U[c, n] = sum_m vT[m, c]^T expS^T[m, n] directly in PSUM across
the 32 m-blocks -- no transposes anywhere. Denominator d[n] = sum_m expS^T via
a ones[128,1] matmul accumulated in PSUM. Final: out = xq + (gamma/d)*U + gamma*bv.

Precision: S path (projections + S matmul) in float32r (TF32-like; moving dim
512 >= 256 runs at full PE rate), expS^T/vT in bf16 (measured to contribute
negligible error), all accumulation in f32 PSUM.
"""

import sys

if "/opt/trn_rl_repo" not in sys.path:
    sys.path.insert(0, "/opt/trn_rl_repo")

import numpy as np

B = 4
C = 256
HID = 64
N = 4096          # H*W
P = 128           # SBUF partitions
NT = 512          # n-tile (matmul moving free dim)
N_NT = N // NT    # 8
MB = 128          # m-block (PV contraction tile)
N_MB = N // MB    # 32
CA = C // P       # 2 c-chunks / c-blocks

_CACHE = {}


def _build_program():
    import concourse.bass as bass
    import concourse.mybir as mybir
    from concourse import tile

    f32 = mybir.dt.float32
    f32r = mybir.dt.float32r
    bf16 = mybir.dt.bfloat16
    AF = mybir.ActivationFunctionType

    nc = bass.Bass("TRN2", target_bir_lowering=False, debug=False)

    xq_d = nc.dram_tensor("xq", (C, N), f32r, kind="ExternalInput")
    xkv_d = nc.dram_tensor("xkv", (C, N), f32r, kind="ExternalInput")
    wqT_d = nc.dram_tensor("wqT", (C, HID), f32r, kind="ExternalInput")
    wkT_d = nc.dram_tensor("wkT", (C, HID), f32r, kind="ExternalInput")
    wvT_d = nc.dram_tensor("wvT", (C, C), f32r, kind="ExternalInput")
    bq_d = nc.dram_tensor("bq", (HID, 1), f32, kind="ExternalInput")
    bk_d = nc.dram_tensor("bk", (HID, 1), f32, kind="ExternalInput")
    gbv_d = nc.dram_tensor("gbv", (C, 1), f32, kind="ExternalInput")      # gamma * bv
    rgam_d = nc.dram_tensor("rgam", (1, 1), f32, kind="ExternalInput")    # 1 / gamma
    out_d = nc.dram_tensor("out", (C, N), f32, kind="ExternalOutput")

    # c = a*128 + p views
    xq_r = xq_d[:].rearrange("(a p) n -> p a n", p=P)
    xkv_r = xkv_d[:].rearrange("(a p) n -> p a n", p=P)
    wqT_r = wqT_d[:].rearrange("(a p) h -> p a h", p=P)
    wkT_r = wkT_d[:].rearrange("(a p) h -> p a h", p=P)
    wvT_r = wvT_d[:].rearrange("(a p) c -> p a c", p=P)
    gbv_r = gbv_d[:].rearrange("(a p) one -> p (a one)", p=P)
    out_r = out_d[:].rearrange("(a p) n -> p a n", p=P)

    with tile.TileContext(nc) as tc:
        with (
            tc.tile_pool(name="const", bufs=1) as const,
            tc.tile_pool(name="xin", bufs=1) as xin,
            tc.tile_pool(name="qk", bufs=1) as qk,
            tc.tile_pool(name="vtp", bufs=1) as vtp,
            tc.tile_pool(name="work", bufs=3) as work,
            tc.tile_pool(name="ep", bufs=2) as ep,
            tc.tile_pool(name="dram", bufs=2, space="DRAM") as dram,
            tc.tile_pool(name="psum", bufs=1, space="PSUM") as psum,
        ):
            # ---- constants / weights ----
            wq_sb = const.tile([P, CA, HID], f32r, tag="wq")
            nc.sync.dma_start(wq_sb[:], wqT_r)
            wk_sb = const.tile([P, CA, HID], f32r, tag="wk")
            nc.sync.dma_start(wk_sb[:], wkT_r)
            wv_sb = const.tile([P, CA, C], f32r, tag="wv")
            nc.sync.dma_start(wv_sb[:], wvT_r)
            bq_sb = const.tile([HID, 1], f32, tag="bq")
            nc.sync.dma_start(bq_sb[:], bq_d[:])
            bk_sb = const.tile([HID, 1], f32, tag="bk")
            nc.sync.dma_start(bk_sb[:], bk_d[:])
            gbv_sb = const.tile([P, CA], f32, tag="gbv")
            nc.sync.dma_start(gbv_sb[:], gbv_r)
            rgam_sb = const.tile([1, 1], f32, tag="rgam")
            nc.sync.dma_start(rgam_sb[:], rgam_d[:])
            ones_sb = const.tile([P, 1], bf16, tag="ones")
            nc.vector.memset(ones_sb[:], 1.0)

            # ---- x loads (chunked for DMA/compute overlap) ----
            xq_sb = xin.tile([P, CA, N], f32r, tag="xq")
            xkv_sb = xin.tile([P, CA, N], f32r, tag="xkv")
            NCH = 1024
            for a in range(CA):
                for h in range(N // NCH):
                    sl = slice(h * NCH, (h + 1) * NCH)
                    nc.sync.dma_start(xkv_sb[:, a, sl], xkv_r[:, a, sl])
            for a in range(CA):
                for h in range(N // NCH):
                    sl = slice(h * NCH, (h + 1) * NCH)
                    nc.sync.dma_start(xq_sb[:, a, sl], xq_r[:, a, sl])

            # ---- projections ----
            # q/k stored twice (rows 0-63 and 64-127) so the K=64 S-matmuls
            # can be row-paired into both halves of the PE array.
            q_sb = qk.tile([P, N], f32r, tag="q")
            k_sb = qk.tile([P, N], f32r, tag="k")
            # k projection first: it only needs xkv, which is DMA'd first
            for nt in range(N_NT):
                ntsl = slice(nt * NT, (nt + 1) * NT)
                kp = psum.tile([P, NT], f32, tag="st", bufs=3)
                for a in range(CA):
                    nc.tensor.matmul(
                        kp[:HID, :],
                        lhsT=wk_sb[:, a, :],
                        rhs=xkv_sb[:, a, ntsl],
                        start=(a == 0),
                        stop=(a == CA - 1),
                    )
                nc.vector.tensor_scalar_add(k_sb[0:HID, ntsl], kp[:HID, :], bk_sb[:])
                nc.vector.tensor_scalar_add(k_sb[HID:P, ntsl], kp[:HID, :], bk_sb[:])

            # bf16 copies of xkv/Wv for the vT projection (bf16 weight loads
            # get FWL; v precision contributes negligibly to the result)
            wvb_sb = const.tile([P, CA, C], bf16, tag="wvb")
            for a in range(CA):
                nc.vector.tensor_copy(wvb_sb[:, a, :], wv_sb[:, a, :])
            xkvb_sb = xin.tile([P, CA, N], bf16, tag="xkvb")
            for a in range(CA):
                for h in range(N // NCH):
                    sl = slice(h * NCH, (h + 1) * NCH)
                    nc.vector.tensor_copy(xkvb_sb[:, a, sl], xkv_sb[:, a, sl])

            # vT[m, c] = sum_c_in xkv[c_in, m] * WvT[c_in, c], stored bf16
            vt_sb = vtp.tile([P, N_MB, C], bf16, tag="vt")
            for mb in range(N_MB):
                msl = slice(mb * MB, (mb + 1) * MB)
                vp = psum.tile([P, NT], f32, tag="st", bufs=3)
                for a in range(CA):
                    nc.tensor.matmul(
                        vp[:, :C],
                        lhsT=xkvb_sb[:, a, msl],
                        rhs=wvb_sb[:, a, :],
                        start=(a == 0),
                        stop=(a == CA - 1),
                    )
                nc.vector.tensor_copy(vt_sb[:, mb, :], vp[:, :C])

            # q projection (needs xq, which lands after xkv)
            for nt in range(N_NT):
                ntsl = slice(nt * NT, (nt + 1) * NT)
                qp = psum.tile([P, NT], f32, tag="st", bufs=3)
                for a in range(CA):
                    nc.tensor.matmul(
                        qp[:HID, :],
                        lhsT=wq_sb[:, a, :],
                        rhs=xq_sb[:, a, ntsl],
                        start=(a == 0),
                        stop=(a == CA - 1),
                    )
                nc.vector.tensor_scalar_add(q_sb[0:HID, ntsl], qp[:HID, :], bq_sb[:])
                nc.vector.tensor_scalar_add(q_sb[HID:P, ntsl], qp[:HID, :], bq_sb[:])

            # ---- attention ----
            for nt in range(N_NT):
                ntsl = slice(nt * NT, (nt + 1) * NT)
                u0 = psum.tile([P, NT], f32, tag="u", bufs=4, name=f"u0_{nt}")
                u1 = psum.tile([P, NT], f32, tag="u", bufs=4, name=f"u1_{nt}")
                dp = psum.tile([1, NT], f32, tag="dd", bufs=1, name=f"dp_{nt}")
                exs = []     # exp tiles awaiting the d sum tree
                n_d = 0      # d-matmuls issued for this n-tile
                for mb in range(N_MB):
                    msl = slice(mb * MB, (mb + 1) * MB)
                    # row-paired S matmul: even m-blocks use PE rows 0-63,
                    # odd ones rows 64-127 (concurrent via tile_position)
                    half = slice(0, HID) if mb % 2 == 0 else slice(HID, P)
                    stp = psum.tile([P, NT], f32, tag="st", bufs=3, name=f"stp_{nt}_{mb}")
                    nc.tensor.matmul(
                        stp[:],
                        lhsT=k_sb[half, msl],
                        rhs=q_sb[half, ntsl],
                        start=True,
                        stop=True,
                    )
                    ex = work.tile([P, NT], bf16, tag="expst", name=f"ex_{nt}_{mb}")
                    nc.scalar.activation(ex[:], stp[:], AF.Exp)
                    first, last = (mb == 0), (mb == N_MB - 1)
                    nc.tensor.matmul(
                        u0[:], lhsT=vt_sb[:, mb, 0:P], rhs=ex[:], start=first, stop=last
                    )
                    nc.tensor.matmul(
                        u1[:], lhsT=vt_sb[:, mb, P:C], rhs=ex[:], start=first, stop=last
                    )
                    # denominator: bf16 binary-counter sum tree on DVE; one
                    # ones-matmul per 8 m-blocks accumulated into dp
                    t_, lvl = ex, 0
                    while exs and exs[-1][0] == lvl:
                        _, prev = exs.pop()
                        s_ = work.tile(
                            [P, NT], bf16, tag=f"dsum{lvl}", bufs=2,
                            name=f"ds_{nt}_{mb}_{lvl}",
                        )
                        nc.vector.tensor_add(s_[:], prev[:], t_[:])
                        t_, lvl = s_, lvl + 1
                    exs.append((lvl, t_))
                    if (mb + 1) % 8 == 0:
                        lvl8, a8 = exs.pop()
                        assert lvl8 == 3 and not exs
                        n_d += 1
                        nc.tensor.matmul(
                            dp[:], lhsT=ones_sb[:], rhs=a8[:],
                            start=(n_d == 1), stop=(n_d == N_MB // 8),
                        )
                # epilogue: out[c, n] = xq + (gamma/d[n]) * U[c, n] + gamma*bv[c]
                rd = ep.tile([1, NT], f32, tag="rd", name=f"rd_{nt}")
                nc.vector.tensor_scalar_mul(rd[:], dp[:], rgam_sb[:])   # d/gamma
                grd = ep.tile([1, NT], f32, tag="grd", name=f"grd_{nt}")
                nc.vector.reciprocal(grd[:], rd[:])                     # gamma/d
                # broadcast [1, NT] -> [P, NT] via DRAM roundtrip (DRAM APs
                # allow partition-stride-0 reads; SBUF ones don't)
                dscr = dram.tile([1, NT], f32, tag="dscr", name=f"dscr_{nt}")
                nc.sync.dma_start(dscr[:], grd[:])
                rdb = ep.tile([P, NT], f32, tag="rdb", name=f"rdb_{nt}")
                nc.sync.dma_start(rdb[:], dscr[:].broadcast_to((P, NT)))
                for cb, u in ((0, u0), (1, u1)):
                    t = ep.tile([P, NT], f32, tag="t", name=f"t_{nt}_{cb}")
                    nc.vector.tensor_mul(t[:], u[:], rdb[:])
                    o = ep.tile([P, NT], f32, tag="o", name=f"o_{nt}_{cb}")
                    nc.vector.scalar_tensor_tensor(
                        o[:],
                        in0=t[:],
                        scalar=gbv_sb[:, cb : cb + 1],
                        in1=xq_sb[:, cb, ntsl],
                        op0=mybir.AluOpType.add,
                        op1=mybir.AluOpType.add,
                    )
                    nc.sync.dma_start(out_r[:, cb, ntsl], o[:])

    return nc


def _split_excess_waits(nc):
    """The pinned walrus build only encodes 1 sync-wait per instruction;
    newer concourse attaches more. Hoist excess waits onto same-engine NoOps
    inserted immediately before the over-limit instruction (semantically
    identical: same engine, same program position)."""
    import concourse.mybir as mybir
    import bass_rust

    ctr = 0
    for bbl in nc.m.functions[0].blocks:
        il = bbl.instructions
        i = 0
        while i < len(il):
            inst = il[i]
            si = inst.sync_info
            limit = 1
            if si is not None and len(si.on_wait) > limit:
                waits = list(si.on_wait)
                extra = waits[limit:]
                for j in range(0, len(extra), 1):
                    nop = mybir.InstNoOp(name=f"I-wsplit-{ctr}", ins=[], outs=[])
                    ctr += 1
                    nop.engine = inst.engine
                    nop.sync_info = bass_rust.SyncInfo(
                        on_wait=[extra[j]], on_update=[]
                    )
                    il.insert(i, nop)
                    i += 1
                si.on_wait = waits[:limit]
                inst.sync_info = si
            i += 1
    return ctr


def _get_program():
    if "nc" not in _CACHE:
        _CACHE["nc"] = _build_program()
    return _CACHE["nc"]


def _get_program_hw():
    """Program with the walrus sync-wait workaround applied (breaks CoreSim's
    race detector, so only applied for hardware runs)."""
    nc = _get_program()
    if not _CACHE.get("split_done"):
        _split_excess_waits(nc)
        _CACHE["split_done"] = True
    return nc


def _make_in_maps(x1, x2, Wq, bq, Wk, bk, Wv, bv, gamma):
    g = float(np.asarray(gamma).reshape(-1)[0])
    shared = {
        "wqT": np.ascontiguousarray(Wq.T, dtype=np.float32),
        "wkT": np.ascontiguousarray(Wk.T, dtype=np.float32),
        "wvT": np.ascontiguousarray(Wv.T, dtype=np.float32),
        "bq": np.asarray(bq, dtype=np.float32).reshape(HID, 1),
        "bk": np.asarray(bk, dtype=np.float32).reshape(HID, 1),
        "gbv": (g * np.asarray(bv, dtype=np.float32)).reshape(C, 1),
        "rgam": np.array([[1.0 / g if g != 0.0 else 0.0]], dtype=np.float32),
    }
    in_maps = []
    for d in range(2):
        src_q, src_kv = (x1, x2) if d == 0 else (x2, x1)
        for b in range(B):
            in_maps.append(
                {
                    "xq": np.ascontiguousarray(src_q[b].reshape(C, N), dtype=np.float32),
                    "xkv": np.ascontiguousarray(src_kv[b].reshape(C, N), dtype=np.float32),
                    **shared,
                }
            )
    return in_maps


def kernel(x1, x2, Wq, bq, Wk, bk, Wv, bv, gamma, _want_results=False):
    x1 = np.asarray(x1, dtype=np.float32)
    x2 = np.asarray(x2, dtype=np.float32)
    nc = _get_program_hw()
    in_maps = _make_in_maps(x1, x2, Wq, bq, Wk, bk, Wv, bv, gamma)

    from concourse.bass_utils import run_bass_kernel_spmd

    res = run_bass_kernel_spmd(nc, in_maps, core_ids=list(range(2 * B)))
    outs = [r["out"].reshape(C, 64, 64) for r in res.results]
    out1 = np.stack(outs[:B]).astype(np.float32)
    out2 = np.stack(outs[B:]).astype(np.float32)
    if _want_results:
        return (out1, out2), res
    return (out1, out2)


# revision 20
# speedup vs baseline: 1.5801x; 1.1233x over previous
"""Trainium2 Bass kernel for nn_CrossModalAttention.

Problem: bidirectional cross-attention between two (B, C, H, W) feature maps.
  B=4, C=256, H=W=64 -> N=4096 pixels, HID=64.
  For each direction:  q = Wq@xq, k = Wk@xkv, v = Wv@xkv (1x1 convs),
  attn = softmax_m(q^T k), out = xq + gamma * (v @ attn^T).

Sharding: 2 directions x 4 batches = 8 independent units, one per NeuronCore.

Per-core kernel layout trick: compute S^T tiles [m(part)=128, n(free)=512] via
matmul(lhsT=k_tile, rhs=q_tile) (contraction over HID=64 on partitions), exp on
ScalarE (logits are bounded ~ +-56, so exp in f32 needs no max-subtraction),
then accumulate U[c, n] = sum_m vT[m, c]^T expS^T[m, n] directly in PSUM across
the 32 m-blocks -- no transposes anywhere. Denominator d[n] = sum_m expS^T via
a ones[128,1] matmul accumulated in PSUM. Final: out = xq + (gamma/d)*U + gamma*bv.

Precision: S path (projections + S matmul) in float32r (TF32-like; moving dim
512 >= 256 runs at full PE rate), expS^T/vT in bf16 (measured to contribute
negligible error), all accumulation in f32 PSUM.
"""

import sys

if "/opt/trn_rl_repo" not in sys.path:
    sys.path.insert(0, "/opt/trn_rl_repo")

import numpy as np

B = 4
C = 256
HID = 64
N = 4096          # H*W
P = 128           # SBUF partitions
NT = 512          # n-tile (matmul moving free dim)
N_NT = N // NT    # 8
MB = 128          # m-block (PV contraction tile)
N_MB = N // MB    # 32
CA = C // P       # 2 c-chunks / c-blocks

_CACHE = {}


def _build_program():
    import concourse.bass as bass
    import concourse.mybir as mybir
    from concourse import tile

    f32 = mybir.dt.float32
    f32r = mybir.dt.float32r
    bf16 = mybir.dt.bfloat16
    AF = mybir.ActivationFunctionType

    nc = bass.Bass("TRN2", target_bir_lowering=False, debug=False)

    xq_d = nc.dram_tensor("xq", (C, N), f32r, kind="ExternalInput")
    xkv_d = nc.dram_tensor("xkv", (C, N), f32r, kind="ExternalInput")
    wqT_d = nc.dram_tensor("wqT", (C, HID), f32r, kind="ExternalInput")
    wkT_d = nc.dram_tensor("wkT", (C, HID), f32r, kind="ExternalInput")
    wvT_d = nc.dram_tensor("wvT", (C, C), f32r, kind="ExternalInput")
    bq_d = nc.dram_tensor("bq", (HID, 1), f32, kind="ExternalInput")
    bk_d = nc.dram_tensor("bk", (HID, 1), f32, kind="ExternalInput")
    gbv_d = nc.dram_tensor("gbv", (C, 1), f32, kind="ExternalInput")      # gamma * bv
    rgam_d = nc.dram_tensor("rgam", (1, 1), f32, kind="ExternalInput")    # 1 / gamma
    out_d = nc.dram_tensor("out", (C, N), f32, kind="ExternalOutput")

    # c = a*128 + p views
    xq_r = xq_d[:].rearrange("(a p) n -> p a n", p=P)
    xkv_r = xkv_d[:].rearrange("(a p) n -> p a n", p=P)
    wqT_r = wqT_d[:].rearrange("(a p) h -> p a h", p=P)
    wkT_r = wkT_d[:].rearrange("(a p) h -> p a h", p=P)
    wvT_r = wvT_d[:].rearrange("(a p) c -> p a c", p=P)
    gbv_r = gbv_d[:].rearrange("(a p) one -> p (a one)", p=P)
    out_r = out_d[:].rearrange("(a p) n -> p a n", p=P)

    with tile.TileContext(nc) as tc:
        with (
            tc.tile_pool(name="const", bufs=1) as const,
            tc.tile_pool(name="xin", bufs=1) as xin,
            tc.tile_pool(name="qk", bufs=1) as qk,
            tc.tile_pool(name="vtp", bufs=1) as vtp,
            tc.tile_pool(name="work", bufs=3) as work,
            tc.tile_pool(name="ep", bufs=2) as ep,
            tc.tile_pool(name="dram", bufs=2, space="DRAM") as dram,
            tc.tile_pool(name="psum", bufs=1, space="PSUM") as psum,
        ):
            # ---- constants / weights ----
            wq_sb = const.tile([P, CA, HID], f32r, tag="wq")
            nc.sync.dma_start(wq_sb[:], wqT_r)
            wk_sb = const.tile([P, CA, HID], f32r, tag="wk")
            nc.sync.dma_start(wk_sb[:], wkT_r)
            wv_sb = const.tile([P, CA, C], f32r, tag="wv")
            nc.sync.dma_start(wv_sb[:], wvT_r)
            bq_sb = const.tile([HID, 1], f32, tag="bq")
            nc.sync.dma_start(bq_sb[:], bq_d[:])
            bk_sb = const.tile([HID, 1], f32, tag="bk")
            nc.sync.dma_start(bk_sb[:], bk_d[:])
            gbv_sb = const.tile([P, CA], f32, tag="gbv")
            nc.sync.dma_start(gbv_sb[:], gbv_r)
            rgam_sb = const.tile([1, 1], f32, tag="rgam")
            nc.sync.dma_start(rgam_sb[:], rgam_d[:])
            ones_sb = const.tile([P, 1], bf16, tag="ones")
            nc.vector.memset(ones_sb[:], 1.0)

            # ---- x loads (chunked for DMA/compute overlap) ----
            xq_sb = xin.tile([P, CA, N], f32r, tag="xq")
            xkv_sb = xin.tile([P, CA, N], f32r, tag="xkv")
            NCH = 1024
            for h in range(N // NCH):
                sl = slice(h * NCH, (h + 1) * NCH)
                for a in range(CA):
                    nc.sync.dma_start(xkv_sb[:, a, sl], xkv_r[:, a, sl])
            for h in range(N // NCH):
                sl = slice(h * NCH, (h + 1) * NCH)
                for a in range(CA):
                    nc.sync.dma_start(xq_sb[:, a, sl], xq_r[:, a, sl])

            # ---- projections ----
            # q/k stored twice (rows 0-63 and 64-127) so the K=64 S-matmuls
            # can be row-paired into both halves of the PE array.
            q_sb = qk.tile([P, N], f32r, tag="q")
            k_sb = qk.tile([P, N], f32r, tag="k")
            # k projection first: it only needs xkv, which is DMA'd first
            for nt in range(N_NT):
                ntsl = slice(nt * NT, (nt + 1) * NT)
                kp = psum.tile([P, NT], f32, tag="st", bufs=3)
                for a in range(CA):
                    nc.tensor.matmul(
                        kp[:HID, :],
                        lhsT=wk_sb[:, a, :],
                        rhs=xkv_sb[:, a, ntsl],
                        start=(a == 0),
                        stop=(a == CA - 1),
                    )
                nc.vector.tensor_scalar_add(k_sb[0:HID, ntsl], kp[:HID, :], bk_sb[:])
                nc.vector.tensor_scalar_add(k_sb[HID:P, ntsl], kp[:HID, :], bk_sb[:])

            # bf16 copies of xkv/Wv for the vT projection (bf16 weight loads
            # get FWL; v precision contributes negligibly to the result)
            wvb_sb = const.tile([P, CA, C], bf16, tag="wvb")
            for a in range(CA):
                nc.vector.tensor_copy(wvb_sb[:, a, :], wv_sb[:, a, :])
            xkvb_sb = xin.tile([P, CA, N], bf16, tag="xkvb")
            for a in range(CA):
                for h in range(N // NCH):
                    sl = slice(h * NCH, (h + 1) * NCH)
                    nc.vector.tensor_copy(xkvb_sb[:, a, sl], xkv_sb[:, a, sl])

            # vT[m, c] = sum_c_in xkv[c_in, m] * WvT[c_in, c], stored bf16
            vt_sb = vtp.tile([P, N_MB, C], bf16, tag="vt")
            for mb in range(N_MB):
                msl = slice(mb * MB, (mb + 1) * MB)
                vp = psum.tile([P, NT], f32, tag="st", bufs=3)
                for a in range(CA):
                    nc.tensor.matmul(
                        vp[:, :C],
                        lhsT=xkvb_sb[:, a, msl],
                        rhs=wvb_sb[:, a, :],
                        start=(a == 0),
                        stop=(a == CA - 1),
                    )
                nc.vector.tensor_copy(vt_sb[:, mb, :], vp[:, :C])

            # q projection (needs xq, which lands after xkv)
            for nt in range(N_NT):
                ntsl = slice(nt * NT, (nt + 1) * NT)
                qp = psum.tile([P, NT], f32, tag="st", bufs=3)
                for a in range(CA):
                    nc.tensor.matmul(
                        qp[:HID, :],
                        lhsT=wq_sb[:, a, :],
                        rhs=xq_sb[:, a, ntsl],
                        start=(a == 0),
                        stop=(a == CA - 1),
                    )
                nc.vector.tensor_scalar_add(q_sb[0:HID, ntsl], qp[:HID, :], bq_sb[:])
                nc.vector.tensor_scalar_add(q_sb[HID:P, ntsl], qp[:HID, :], bq_sb[:])

            # ---- attention ----
            DG = 8           # m-blocks per denominator group
            N_DG = N_MB // DG

            def _epilogue(nt, u0, u1, dp, final_dmm):
                # out[c, n] = xq + (gamma/d[n]) * U[c, n] + gamma*bv[c]
                ntsl = slice(nt * NT, (nt + 1) * NT)
                final_dmm()
                rd = ep.tile([1, NT], f32, tag="rd", name=f"rd_{nt}")
                nc.vector.tensor_scalar_mul(rd[:], dp[:], rgam_sb[:])   # d/gamma
                grd = ep.tile([1, NT], f32, tag="grd", name=f"grd_{nt}")
                nc.vector.reciprocal(grd[:], rd[:])                     # gamma/d
                # broadcast [1, NT] -> [P, NT] via DRAM roundtrip (DRAM APs
                # allow partition-stride-0 reads; SBUF ones don't)
                dscr = dram.tile([1, NT], f32, tag="dscr", name=f"dscr_{nt}")
                nc.sync.dma_start(dscr[:], grd[:])
                rdb = ep.tile([P, NT], f32, tag="rdb", name=f"rdb_{nt}")
                nc.sync.dma_start(rdb[:], dscr[:].broadcast_to((P, NT)))
                for cb, u in ((0, u0), (1, u1)):
                    t = ep.tile([P, NT], f32, tag="t", name=f"t_{nt}_{cb}")
                    nc.vector.tensor_mul(t[:], u[:], rdb[:])
                    o = ep.tile([P, NT], f32, tag="o", name=f"o_{nt}_{cb}")
                    nc.vector.scalar_tensor_tensor(
                        o[:],
                        in0=t[:],
                        scalar=gbv_sb[:, cb : cb + 1],
                        in1=xq_sb[:, cb, ntsl],
                        op0=mybir.AluOpType.add,
                        op1=mybir.AluOpType.add,
                    )
                    nc.sync.dma_start(out_r[:, cb, ntsl], o[:])

            pending_epi = [None]  # previous n-tile's epilogue, deferred so its
            # final d-matmul doesn't stall the in-order PE queue at the boundary

            for nt in range(N_NT):
                ntsl = slice(nt * NT, (nt + 1) * NT)
                u0 = psum.tile([P, NT], f32, tag="u", bufs=4, name=f"u0_{nt}")
                u1 = psum.tile([P, NT], f32, tag="u", bufs=4, name=f"u1_{nt}")
                dp = psum.tile([1, NT], f32, tag="dd", bufs=1, name=f"dp_{nt}")
                acc = None   # running bf16 partial-sum for the current d group
                n_d = 0      # d-matmuls issued for this n-tile
                for mb in range(N_MB):
                    msl = slice(mb * MB, (mb + 1) * MB)
                    # row-paired S matmul: even m-blocks use PE rows 0-63,
                    # odd ones rows 64-127 (concurrent via tile_position)
                    half = slice(0, HID) if mb % 2 == 0 else slice(HID, P)
                    stp = psum.tile([P, NT], f32, tag="st", bufs=3, name=f"stp_{nt}_{mb}")
                    nc.tensor.matmul(
                        stp[:],
                        lhsT=k_sb[half, msl],
                        rhs=q_sb[half, ntsl],
                        start=True,
                        stop=True,
                    )
                    ex = work.tile([P, NT], bf16, tag="expst", name=f"ex_{nt}_{mb}")
                    nc.scalar.activation(ex[:], stp[:], AF.Exp)
                    first, last = (mb == 0), (mb == N_MB - 1)
                    nc.tensor.matmul(
                        u0[:], lhsT=vt_sb[:, mb, 0:P], rhs=ex[:], start=first, stop=last
                    )
                    nc.tensor.matmul(
                        u1[:], lhsT=vt_sb[:, mb, P:C], rhs=ex[:], start=first, stop=last
                    )
                    if mb == 3 and pending_epi[0] is not None:
                        pending_epi[0]()
                        pending_epi[0] = None
                    # denominator: running bf16 sum on DVE; one ones-matmul
                    # per DG m-blocks accumulated into dp
                    if mb % DG == 0:
                        acc = ex
                    else:
                        s_ = work.tile(
                            [P, NT], bf16, tag=f"dacc{mb % 2}", bufs=3,
                            name=f"ds_{nt}_{mb}",
                        )
                        nc.vector.tensor_add(s_[:], acc[:], ex[:])
                        acc = s_
                    if (mb + 1) % DG == 0:
                        n_d += 1
                        a8, nd = acc, n_d
                        def _dmm(a8=a8, nd=nd, dp=dp):
                            nc.tensor.matmul(
                                dp[:], lhsT=ones_sb[:], rhs=a8[:],
                                start=(nd == 1), stop=(nd == N_DG),
                            )
                        if nd == N_DG:
                            pending_epi[0] = (
                                lambda nt=nt, u0=u0, u1=u1, dp=dp, dmm=_dmm:
                                _epilogue(nt, u0, u1, dp, dmm)
                            )
                        else:
                            _dmm()
                        acc = None
            pending_epi[0]()

    return nc


def _split_excess_waits(nc):
    """The pinned walrus build only encodes 1 sync-wait per instruction;
    newer concourse attaches more. Hoist excess waits onto same-engine NoOps
    inserted immediately before the over-limit instruction (semantically
    identical: same engine, same program position)."""
    import concourse.mybir as mybir
    import bass_rust

    ctr = 0
    for bbl in nc.m.functions[0].blocks:
        il = bbl.instructions
        i = 0
        while i < len(il):
            inst = il[i]
            si = inst.sync_info
            limit = 1
            if si is not None and len(si.on_wait) > limit:
                waits = list(si.on_wait)
                extra = waits[limit:]
                for j in range(0, len(extra), 1):
                    nop = mybir.InstNoOp(name=f"I-wsplit-{ctr}", ins=[], outs=[])
                    ctr += 1
                    nop.engine = inst.engine
                    nop.sync_info = bass_rust.SyncInfo(
                        on_wait=[extra[j]], on_update=[]
                    )
                    il.insert(i, nop)
                    i += 1
                si.on_wait = waits[:limit]
                inst.sync_info = si
            i += 1
    return ctr


def _get_program():
    if "nc" not in _CACHE:
        _CACHE["nc"] = _build_program()
    return _CACHE["nc"]


def _get_program_hw():
    """Program with the walrus sync-wait workaround applied (breaks CoreSim's
    race detector, so only applied for hardware runs)."""
    nc = _get_program()
    if not _CACHE.get("split_done"):
        _split_excess_waits(nc)
        _CACHE["split_done"] = True
    return nc


def _make_in_maps(x1, x2, Wq, bq, Wk, bk, Wv, bv, gamma):
    g = float(np.asarray(gamma).reshape(-1)[0])
    shared = {
        "wqT": np.ascontiguousarray(Wq.T, dtype=np.float32),
        "wkT": np.ascontiguousarray(Wk.T, dtype=np.float32),
        "wvT": np.ascontiguousarray(Wv.T, dtype=np.float32),
        "bq": np.asarray(bq, dtype=np.float32).reshape(HID, 1),
        "bk": np.asarray(bk, dtype=np.float32).reshape(HID, 1),
        "gbv": (g * np.asarray(bv, dtype=np.float32)).reshape(C, 1),
        "rgam": np.array([[1.0 / g if g != 0.0 else 0.0]], dtype=np.float32),
    }
    in_maps = []
    for d in range(2):
        src_q, src_kv = (x1, x2) if d == 0 else (x2, x1)
        for b in range(B):
            in_maps.append(
                {
                    "xq": np.ascontiguousarray(src_q[b].reshape(C, N), dtype=np.float32),
                    "xkv": np.ascontiguousarray(src_kv[b].reshape(C, N), dtype=np.float32),
                    **shared,
                }
            )
    return in_maps


def kernel(x1, x2, Wq, bq, Wk, bk, Wv, bv, gamma, _want_results=False):
    x1 = np.asarray(x1, dtype=np.float32)
    x2 = np.asarray(x2, dtype=np.float32)
    nc = _get_program_hw()
    in_maps = _make_in_maps(x1, x2, Wq, bq, Wk, bk, Wv, bv, gamma)

    from concourse.bass_utils import run_bass_kernel_spmd

    res = run_bass_kernel_spmd(nc, in_maps, core_ids=list(range(2 * B)))
    outs = [r["out"].reshape(C, 64, 64) for r in res.results]
    out1 = np.stack(outs[:B]).astype(np.float32)
    out2 = np.stack(outs[B:]).astype(np.float32)
    if _want_results:
        return (out1, out2), res
    return (out1, out2)
